# revision 7
# baseline (speedup 1.0000x reference)
"""Multi-head attention block (B=4, N=1024, C=1024, H=16, d=64) on 8 TRN2 cores.

Sharding: core = 2*b + hh  (batch b in 0..3, head-half hh in 0..1 -> 8 heads/core).
Each core computes qkv projection for its 8 heads, attention, and a partial
output projection (its 512 rows of w_proj). Host sums the two partials per
batch and adds b_proj.

On-chip layout strategy (all matmul inputs in float32r -> 1 cycle/row on PE):
  - x^T fed from host, so Y_qk^T[cols,seq] = (W_qk chunks).T @ x^T chunks gives
    q^T,k^T directly; Y_v[seq,vcols] = (x^T chunks).T @ W_v gives V naturally.
  - S^T[keys,q] = k^T.T @ q^T (K=64 contraction), exp via ACT (scale=1/8 folded)
  - AV with V augmented by a ones column: one PSUM accumulation yields both
    att^T[64,q] and the softmax denominators (row 64).
  - normalize: reciprocal on DVE, broadcast over 64 partitions via a tiny
    PE matmul against a 0/1 pattern, DVE multiply -> att^T (f32r)
  - proj: out[seq,outfeat] = (att^T chunks).T @ w_proj chunks. Partial result
    DMA'd out; host reduces.
"""

import numpy as np

B = 4
N = 1024
C = 1024
H = 16
D = 64
NCORES = 8
SCALE = D ** -0.5


_NC_CACHE = {}


def _build_bass():
    import concourse.mybir as mybir
    from concourse import bacc
    from concourse.tile import TileContext

    dt = mybir.dt
    f32 = dt.float32
    f32r = dt.float32r
    Act = mybir.ActivationFunctionType

    nc = bacc.Bacc(
        "TRN2", target_bir_lowering=False, debug=False, num_devices=NCORES
    )

    # ---- DRAM I/O (per-core shards; host prepares layouts) ----
    xT_d = nc.dram_tensor("xT", [C, N], f32, kind="ExternalInput").ap()
    wqk_d = nc.dram_tensor("wqk", [C, 1024], f32, kind="ExternalInput").ap()
    wv_d = nc.dram_tensor("wv", [C, 512], f32, kind="ExternalInput").ap()
    wp_d = nc.dram_tensor("wp", [512, C], f32, kind="ExternalInput").ap()
    bqk_d = nc.dram_tensor("bqk", [128, 8], f32, kind="ExternalInput").ap()
    bv_d = nc.dram_tensor("bv", [128, 512], f32, kind="ExternalInput").ap()
    ones_d = nc.dram_tensor("ones8", [128, 8], f32, kind="ExternalInput").ap()
    pat_d = nc.dram_tensor("pat", [128, 256], f32, kind="ExternalInput").ap()
    y_d = nc.dram_tensor("y", [N, C], f32, kind="ExternalOutput").ap()

    with TileContext(nc) as tc:
        with tc.tile_pool(name="persist", bufs=1) as persist:
            # persistent SBUF tensors
            yqk = persist.tile([128, 8, N], f32r, tag="yqk")  # q^T,k^T rows
            vst = [
                persist.tile([128, 8, 65], f32r, tag=f"vst{s}", name=f"vst{s}")
                for s in range(8)
            ]  # V-hat per key chunk: [keys128, head, d + ones]
            # softmax denoms: head h -> partition 32*(h//4), free idx h%4
            sums = persist.tile([128, 4, N], f32, tag="sums")
            recr = persist.tile([128, 4, N], f32r, tag="recr")
            bqk_t = persist.tile([128, 8], f32, tag="bqk")
            bv_t = persist.tile([128, 512], f32, tag="bv")
            pat_t = persist.tile([128, 2, 128], f32r, tag="pat")

            nc.sync.dma_start(bqk_t[:], bqk_d)
            nc.sync.dma_start(bv_t[:], bv_d)
            nc.gpsimd.dma_start(pat_t[:], pat_d.rearrange("p (a b) -> p a b", a=2))
            for s in range(8):
                nc.gpsimd.dma_start(vst[s][:, :, 64], ones_d)

            # ---------------- Phase 1: QKV projection ----------------
            with (
                tc.tile_pool(name="ph1", bufs=1) as ph1,
                tc.tile_pool(name="ps1", bufs=4, space="PSUM") as ps1,
            ):
                xT = ph1.tile([128, 8, N], f32r, tag="xT")
                wqk = ph1.tile([128, 8, 1024], f32r, tag="wqk")
                wv = ph1.tile([128, 8, 512], f32r, tag="wv")
                for k in range(8):
                    nc.gpsimd.dma_start(xT[:, k, :], xT_d[k * 128:(k + 1) * 128, :])
                    nc.gpsimd.dma_start(wqk[:, k, :], wqk_d[k * 128:(k + 1) * 128, :])
                    nc.gpsimd.dma_start(wv[:, k, :], wv_d[k * 128:(k + 1) * 128, :])

                # Y_qk^T [cols, seq]: 8 col-chunks x 2 seq-chunks
                for c in range(8):
                    for s in range(2):
                        pt = ps1.tile([128, 512], f32, tag="mm")
                        for k in range(8):
                            nc.tensor.matmul(
                                pt[:],
                                wqk[:, k, c * 128:(c + 1) * 128],
                                xT[:, k, s * 512:(s + 1) * 512],
                                start=(k == 0),
                                stop=(k == 7),
                            )
                        nc.scalar.activation(
                            yqk[:, c, s * 512:(s + 1) * 512],
                            pt[:],
                            Act.Identity,
                            bias=bqk_t[:, c:c + 1],
                        )

                # Y_v [seq, vcols]: 8 seq-chunks
                for s in range(8):
                    pt = ps1.tile([128, 512], f32, tag="mm")
                    for k in range(8):
                        nc.tensor.matmul(
                            pt[:],
                            xT[:, k, s * 128:(s + 1) * 128],
                            wv[:, k, :],
                            start=(k == 0),
                            stop=(k == 7),
                        )
                    nc.vector.tensor_add(
                        out=vst[s][:, :, 0:64],
                        in0=pt[:].rearrange("p (h d) -> p h d", h=8),
                        in1=bv_t[:].rearrange("p (h d) -> p h d", h=8),
                    )

            # ---------------- Phase 2: attention per head ----------------
            with (
                tc.tile_pool(name="ph2", bufs=1) as ph2,
                tc.tile_pool(name="es_pool", bufs=10) as es_pool,
                tc.tile_pool(name="ps2", bufs=3, space="PSUM") as ps2,
                tc.tile_pool(name="psav", bufs=2, space="PSUM") as psav,
            ):
                attf = ph2.tile([128, 4, N], f32, tag="attf")  # pre-normalize att^T
                attr = ph2.tile([128, 4, N], f32r, tag="attr")  # normalized att^T
                wp = ph2.tile([128, 4, 1024], f32r, tag="wp")
                for c in range(4):
                    nc.gpsimd.dma_start(wp[:, c, :], wp_d[c * 128:(c + 1) * 128, :])

                for h in range(8):
                    cq = h // 2
                    ck = 4 + h // 2
                    p0 = (h % 2) * 64
                    # S^T tiles + exp
                    es = [
                        es_pool.tile([128, N], f32r, tag="es", name=f"es{h}_{kc}")
                        for kc in range(8)
                    ]
                    for kc in range(8):
                        for qc in range(2):
                            pt = ps2.tile([128, 512], f32, tag="s")
                            nc.tensor.matmul(
                                pt[:],
                                yqk[p0:p0 + 64, ck, kc * 128:(kc + 1) * 128],
                                yqk[p0:p0 + 64, cq, qc * 512:(qc + 1) * 512],
                                start=True,
                                stop=True,
                            )
                            nc.scalar.activation(
                                es[kc][:, qc * 512:(qc + 1) * 512],
                                pt[:],
                                Act.Exp,
                                scale=SCALE,
                            )
                    # AV (+ softmax denominator via ones column)
                    for qc in range(2):
                        pav = psav.tile([65, 512], f32, tag="av")
                        for kc in range(8):
                            nc.tensor.matmul(
                                pav[:],
                                vst[kc][:, h, :],
                                es[kc][:, qc * 512:(qc + 1) * 512],
                                start=(kc == 0),
                                stop=(kc == 7),
                            )
                        nc.vector.tensor_copy(
                            sums[32 * (h // 4):32 * (h // 4) + 1, h % 4,
                                 qc * 512:(qc + 1) * 512],
                            pav[64:65, :],
                        )
                        nc.scalar.copy(
                            attf[p0:p0 + 64, h // 2, qc * 512:(qc + 1) * 512],
                            pav[0:64, :],
                        )

                # normalize: recip -> broadcast over partitions -> multiply
                with nc.allow_low_precision(reason="f32r recip for bcast matmul"):
                    for pb_ in (0, 32):
                        nc.vector.reciprocal(
                            recr[pb_:pb_ + 1, :, :],
                            sums[pb_:pb_ + 1, :, :],
                        )
                for c in range(4):
                    for qc in range(2):
                        pbase = 32 * (c // 2)
                        j0 = (2 * c) % 4
                        pb = ps2.tile([128, 512], f32, tag="s")
                        nc.tensor.matmul(
                            pb[:],
                            pat_t[pbase:pbase + 1, 0, :],
                            recr[pbase:pbase + 1, j0, qc * 512:(qc + 1) * 512],
                            start=True,
                            stop=False,
                        )
                        nc.tensor.matmul(
                            pb[:],
                            pat_t[pbase:pbase + 1, 1, :],
                            recr[pbase:pbase + 1, j0 + 1, qc * 512:(qc + 1) * 512],
                            start=False,
                            stop=True,
                        )
                        nc.vector.tensor_mul(
                            out=attr[:, c, qc * 512:(qc + 1) * 512],
                            in0=attf[:, c, qc * 512:(qc + 1) * 512],
                            in1=pb[:],
                        )

                # ---------------- Phase 3: output projection ----------------
                with tc.tile_pool(name="yo_pool", bufs=3) as yo_pool:
                    for st in range(8):
                        for oc in range(2):
                            pt = ps2.tile([128, 512], f32, tag="s")
                            for c in range(4):
                                nc.tensor.matmul(
                                    pt[:],
                                    attr[:, c, st * 128:(st + 1) * 128],
                                    wp[:, c, oc * 512:(oc + 1) * 512],
                                    start=(c == 0),
                                    stop=(c == 3),
                                )
                            yo = yo_pool.tile([128, 512], f32, tag="yo")
                            nc.scalar.copy(yo[:], pt[:])
                            nc.sync.dma_start(
                                y_d[st * 128:(st + 1) * 128,
                                    oc * 512:(oc + 1) * 512],
                                yo[:],
                            )

    nc.compile()
    return nc


def _get_nc():
    if "nc" not in _NC_CACHE:
        _NC_CACHE["nc"] = _build_bass()
    return _NC_CACHE["nc"]


def _shard_inputs(x, w_qkv, b_qkv, w_proj):
    """Build per-core input maps. core = 2*b + hh."""
    pat = np.zeros((1, 256), dtype=np.float32)
    pat[0, 0:64] = 1.0      # mask for partitions 0:64
    pat[0, 128 + 64:] = 1.0  # mask for partitions 64:128
    pat = np.ascontiguousarray(np.broadcast_to(pat, (128, 256)))
    ones8 = np.ones((128, 8), dtype=np.float32)

    in_maps = []
    for core in range(NCORES):
        b = core // 2
        hh = core % 2
        q_sl = slice(hh * 512, (hh + 1) * 512)
        k_sl = slice(1024 + hh * 512, 1024 + (hh + 1) * 512)
        v_sl = slice(2048 + hh * 512, 2048 + (hh + 1) * 512)

        xT = np.ascontiguousarray(x[b].T)
        wqk = np.ascontiguousarray(
            np.concatenate([w_qkv[:, q_sl], w_qkv[:, k_sl]], axis=1)
        )
        wv = np.ascontiguousarray(w_qkv[:, v_sl])
        wp = np.ascontiguousarray(w_proj[hh * 512:(hh + 1) * 512, :])
        bqk = np.ascontiguousarray(
            np.concatenate([b_qkv[q_sl], b_qkv[k_sl]]).reshape(8, 128).T
        )
        bv = np.ascontiguousarray(
            np.broadcast_to(b_qkv[v_sl], (128, 512))
        )
        in_maps.append(
            {
                "xT": xT,
                "wqk": wqk,
                "wv": wv,
                "wp": wp,
                "bqk": bqk,
                "bv": bv,
                "ones8": ones8,
                "pat": pat,
            }
        )
    return in_maps


def kernel(x, w_qkv, b_qkv, w_proj, b_proj):
    from concourse.bass_utils import run_bass_kernel_spmd

    x = np.asarray(x, dtype=np.float32)
    w_qkv = np.asarray(w_qkv, dtype=np.float32)
    b_qkv = np.asarray(b_qkv, dtype=np.float32)
    w_proj = np.asarray(w_proj, dtype=np.float32)
    b_proj = np.asarray(b_proj, dtype=np.float32)

    nc = _get_nc()
    in_maps = _shard_inputs(x, w_qkv, b_qkv, w_proj)
    res = run_bass_kernel_spmd(nc, in_maps, core_ids=list(range(NCORES)))

    out = np.empty((B, N, C), dtype=np.float32)
    for b in range(B):
        out[b] = res.results[2 * b]["y"] + res.results[2 * b + 1]["y"]
    out += b_proj
    return out


# revision 8
# speedup vs baseline: 1.1319x; 1.1319x over previous
"""Multi-head attention block (B=4, N=1024, C=1024, H=16, d=64) on 8 TRN2 cores.

Sharding: core = 2*b + hh  (batch b in 0..3, head-half hh in 0..1 -> 8 heads/core).
Each core computes the qkv projection for its 8 heads, attention, and a partial
output projection (its 512 rows of w_proj). Host sums the two partials per
batch and adds b_proj.

Per-core pipeline (all matmul inputs float32r -> 1 PE cycle/row):
  - x^T fed from host, so Y_qk^T[cols,seq] = (W_qk chunks).T @ x^T chunks gives
    q^T,k^T directly; Y_v[seq,vcols] = (x^T chunks).T @ W_v gives V naturally.
    Zero on-chip transposes.
  - per head pair (heads 2p, 2p+1 at partition bases 0/64): S^T[keys,q] =
    k^T.T @ q^T with K=64; the two heads' matmuls target disjoint PE row
    groups and run concurrently. exp on ACT (scale=1/8 folded in), into f32r.
  - AV with V augmented by a ones column: one PSUM accumulation yields both
    att^T[64,q] and the softmax denominators (row 64). Normalize: DVE
    reciprocal (PSUM row), gpsimd partition_broadcast, DVE multiply -> att^T.
  - proj: out[seq,outfeat] = (att^T chunks).T @ w_proj chunks, DVE evict,
    DMA out. QKV production, attention, and eviction pipeline across engines;
    phases interleave per head pair.
"""

import numpy as np

B = 4
N = 1024
C = 1024
H = 16
D = 64
NCORES = 8
SCALE = D ** -0.5


_NC_CACHE = {}


def _build_bass():
    import concourse.mybir as mybir
    from concourse import bacc
    from concourse.tile import TileContext

    dt = mybir.dt
    f32 = dt.float32
    f32r = dt.float32r
    Act = mybir.ActivationFunctionType

    nc = bacc.Bacc(
        "TRN2",
        target_bir_lowering=False,
        debug=False,
        num_devices=NCORES,
        num_swdge_queues=4,
    )

    # ---- DRAM I/O (per-core shards; host prepares layouts) ----
    xT_d = nc.dram_tensor("xT", [C, N], f32, kind="ExternalInput").ap()
    wqk_d = nc.dram_tensor("wqk", [C, 1024], f32, kind="ExternalInput").ap()
    wv_d = nc.dram_tensor("wv", [C, 512], f32, kind="ExternalInput").ap()
    wp_d = nc.dram_tensor("wp", [512, C], f32, kind="ExternalInput").ap()
    bqk_d = nc.dram_tensor("bqk", [128, 8], f32, kind="ExternalInput").ap()
    bv_d = nc.dram_tensor("bv", [128, 512], f32, kind="ExternalInput").ap()
    ones_d = nc.dram_tensor("ones64", [128, 64], f32, kind="ExternalInput").ap()
    y_d = nc.dram_tensor("y", [N, C], f32, kind="ExternalOutput").ap()

    with TileContext(nc) as tc:
        with (
            tc.tile_pool(name="persist", bufs=1) as persist,
            tc.tile_pool(name="yqk_pool", bufs=2) as yqk_pool,
            tc.tile_pool(name="es_pool", bufs=9) as es_pool,
            tc.tile_pool(name="norm", bufs=4) as norm,
            tc.tile_pool(name="psum", bufs=3, space="PSUM") as ps,
            tc.tile_pool(name="psav", bufs=2, space="PSUM") as psav,
        ):
            # persistent SBUF tensors
            vst = persist.tile([128, 8, 8, 65], f32r, tag="vst")  # [keys128, s, h, d+1]
            attr = persist.tile([128, 4, N], f32r, tag="attr")  # att^T normalized
            bqk_t = persist.tile([128, 8], f32, tag="bqk")
            bv_t = persist.tile([128, 512], f32, tag="bv")

            nc.sync.dma_start(bqk_t[:], bqk_d)
            nc.sync.dma_start(bv_t[:], bv_d)

            with tc.tile_pool(name="ph1", bufs=1) as ph1:
                xT = [
                    ph1.tile([128, N], f32r, tag=f"xT{k}", name=f"xT{k}")
                    for k in range(8)
                ]
                wqk = [
                    ph1.tile([128, 1024], f32r, tag=f"wqk{k}", name=f"wqk{k}")
                    for k in range(8)
                ]
                wv = [
                    ph1.tile([128, 512], f32r, tag=f"wv{k}", name=f"wv{k}")
                    for k in range(8)
                ]
                for k in range(8):
                    nc.gpsimd.dma_start(xT[k][:], xT_d[k * 128:(k + 1) * 128, :])
                    nc.gpsimd.dma_start(wqk[k][:], wqk_d[k * 128:(k + 1) * 128, :])
                for k in range(8):
                    nc.gpsimd.dma_start(wv[k][:], wv_d[k * 128:(k + 1) * 128, :])
                # ones column of V-hat
                nc.gpsimd.dma_start(
                    vst[:, :, :, 64],
                    ones_d.rearrange("p (s h) -> p s h", s=8),
                )

                # ---- Y_v [seq, vcols] ----
                for s in range(8):
                    pv = ps.tile([128, 512], f32, tag="s", name=f"pv{s}")
                    for k in range(8):
                        nc.tensor.matmul(
                            pv[:],
                            xT[k][:, s * 128:(s + 1) * 128],
                            wv[k][:],
                            start=(k == 0),
                            stop=(k == 7),
                        )
                    nc.vector.tensor_add(
                        out=vst[:, s, :, 0:64],
                        in0=pv[:].rearrange("p (h d) -> p h d", h=8),
                        in1=bv_t[:].rearrange("p (h d) -> p h d", h=8),
                    )

                # ---- per head-pair pipeline ----
                for p in range(4):
                    # Y_qk^T for this pair's q-cols (chunk p) and k-cols (4+p)
                    ytiles = {}
                    for cc, tagn in ((p, "yq"), (4 + p, "yk")):
                        pq = ps.tile([128, N], f32, tag="s", name=f"pq{cc}")
                        for s in range(2):
                            for k in range(8):
                                nc.tensor.matmul(
                                    pq[:, s * 512:(s + 1) * 512],
                                    wqk[k][:, cc * 128:(cc + 1) * 128],
                                    xT[k][:, s * 512:(s + 1) * 512],
                                    start=(k == 0),
                                    stop=(k == 7),
                                )
                        yt = yqk_pool.tile(
                            [128, N], f32r, tag=tagn, name=f"{tagn}{p}"
                        )
                        nc.vector.tensor_scalar_add(yt[:], pq[:], bqk_t[:, cc:cc + 1])
                        ytiles[tagn] = yt
                    yq, yk = ytiles["yq"], ytiles["yk"]

                    # S^T + exp, heads 2p (base 0) and 2p+1 (base 64) row-packed
                    es = {
                        (j, kc): es_pool.tile(
                            [128, N], f32r, tag="es", name=f"es{p}_{j}_{kc}"
                        )
                        for j in range(2)
                        for kc in range(8)
                    }
                    for kc in range(8):
                        psj = [
                            ps.tile([128, N], f32, tag="s", name=f"ps{p}_{j}_{kc}")
                            for j in range(2)
                        ]
                        for qc in range(2):
                            for j, p0 in ((0, 0), (1, 64)):
                                nc.tensor.matmul(
                                    psj[j][:, qc * 512:(qc + 1) * 512],
                                    yk[p0:p0 + 64, kc * 128:(kc + 1) * 128],
                                    yq[p0:p0 + 64, qc * 512:(qc + 1) * 512],
                                    start=True,
                                    stop=True,
                                )
                        for j in range(2):
                            nc.scalar.activation(
                                es[(j, kc)][:], psj[j][:], Act.Exp, scale=SCALE
                            )

                    # AV + normalize per head
                    for j, p0 in ((0, 0), (1, 64)):
                        h = 2 * p + j
                        for qc in range(2):
                            pav = psav.tile(
                                [65, 512], f32, tag="av", name=f"pav{h}_{qc}"
                            )
                            for kc in range(8):
                                nc.tensor.matmul(
                                    pav[:],
                                    vst[:, kc, h, :],
                                    es[(j, kc)][:, qc * 512:(qc + 1) * 512],
                                    start=(kc == 0),
                                    stop=(kc == 7),
                                )
                            rc = norm.tile([1, 512], f32, tag="rc", name=f"rc{h}{qc}")
                            nc.vector.reciprocal(rc[:], pav[64:65, :])
                            bc = norm.tile(
                                [64, 512], f32, tag="bc", name=f"bc{h}{qc}"
                            )
                            nc.gpsimd.partition_broadcast(bc[:], rc[0:1, :])
                            nc.vector.tensor_mul(
                                out=attr[p0:p0 + 64, p, qc * 512:(qc + 1) * 512],
                                in0=pav[0:64, :],
                                in1=bc[:],
                            )

            # ---- output projection ----
            with tc.tile_pool(name="proj", bufs=1) as proj:
                wp = [
                    proj.tile([128, 1024], f32r, tag=f"wp{c}", name=f"wp{c}")
                    for c in range(4)
                ]
                for c in range(4):
                    nc.gpsimd.dma_start(wp[c][:], wp_d[c * 128:(c + 1) * 128, :])
                with tc.tile_pool(name="yo_pool", bufs=3) as yo_pool:
                    for st in range(8):
                        po = ps.tile([128, N], f32, tag="s", name=f"po{st}")
                        for oc in range(2):
                            for c in range(4):
                                nc.tensor.matmul(
                                    po[:, oc * 512:(oc + 1) * 512],
                                    attr[:, c, st * 128:(st + 1) * 128],
                                    wp[c][:, oc * 512:(oc + 1) * 512],
                                    start=(c == 0),
                                    stop=(c == 3),
                                )
                        yo = yo_pool.tile([128, N], f32, tag="yo", name=f"yo{st}")
                        nc.vector.tensor_copy(yo[:], po[:])
                        nc.sync.dma_start(y_d[st * 128:(st + 1) * 128, :], yo[:])

    nc.compile()
    return nc


def _get_nc():
    if "nc" not in _NC_CACHE:
        _NC_CACHE["nc"] = _build_bass()
    return _NC_CACHE["nc"]


def _shard_inputs(x, w_qkv, b_qkv, w_proj):
    """Build per-core input maps. core = 2*b + hh."""
    ones64 = np.ones((128, 64), dtype=np.float32)

    in_maps = []
    for core in range(NCORES):
        b = core // 2
        hh = core % 2
        q_sl = slice(hh * 512, (hh + 1) * 512)
        k_sl = slice(1024 + hh * 512, 1024 + (hh + 1) * 512)
        v_sl = slice(2048 + hh * 512, 2048 + (hh + 1) * 512)

        xT = np.ascontiguousarray(x[b].T)
        wqk = np.ascontiguousarray(
            np.concatenate([w_qkv[:, q_sl], w_qkv[:, k_sl]], axis=1)
        )
        wv = np.ascontiguousarray(w_qkv[:, v_sl])
        wp = np.ascontiguousarray(w_proj[hh * 512:(hh + 1) * 512, :])
        bqk = np.ascontiguousarray(
            np.concatenate([b_qkv[q_sl], b_qkv[k_sl]]).reshape(8, 128).T
        )
        bv = np.ascontiguousarray(np.broadcast_to(b_qkv[v_sl], (128, 512)))
        in_maps.append(
            {
                "xT": xT,
                "wqk": wqk,
                "wv": wv,
                "wp": wp,
                "bqk": bqk,
                "bv": bv,
                "ones64": ones64,
            }
        )
    return in_maps


def kernel(x, w_qkv, b_qkv, w_proj, b_proj):
    from concourse.bass_utils import run_bass_kernel_spmd

    x = np.asarray(x, dtype=np.float32)
    w_qkv = np.asarray(w_qkv, dtype=np.float32)
    b_qkv = np.asarray(b_qkv, dtype=np.float32)
    w_proj = np.asarray(w_proj, dtype=np.float32)
    b_proj = np.asarray(b_proj, dtype=np.float32)

    nc = _get_nc()
    in_maps = _shard_inputs(x, w_qkv, b_qkv, w_proj)
    res = run_bass_kernel_spmd(nc, in_maps, core_ids=list(range(NCORES)))

    out = np.empty((B, N, C), dtype=np.float32)
    for b in range(B):
        out[b] = res.results[2 * b]["y"] + res.results[2 * b + 1]["y"]
    out += b_proj
    return out
